# revision 6
# baseline (speedup 1.0000x reference)
"""Trainium2 Bass kernel for nn_BehaviorEngine (gnn_message_passing).

Per-cell computation over a 1024x1024 grid, D=32:
  cube = A^3 (elementwise); self_pattern = cube/(||cube||+eps)*||A||
  cwp  = A + 0.1*self_pattern
  x    = [cwp ; potential]  (33)
  h    = relu(x @ W1 + b1)  (64)
  out  = h @ W2 + b2 + 0.3*A

Sharding: pure data parallel over H across 8 cores (131072 cells/core).

v3 dataflow (per core, 16 blocks of 8192 cells):
  - single HBM read: fp32->fp16 cast-DMA (gpsimd queue); fp32 A never loaded
  - elementwise in natural layout [128 cells, j, d] on DVE packed-fp16
    (2x DVE mode); d-reductions via 5-level add-trees (TensorReduce is ~10x
    slower on HW); s6 tree in bf16 for range
  - per-cell scale tmp = cube*rb on GPSIMD with a stride-0 broadcast view
    (measured fast on Q7; the same op on DVE is 5x slower)
  - cells paired 2-per-matmul: x-tile [128, 68] -> PE transpose -> [68,128],
    mm1 with block-diagonal [W1;b1] pairs -> hidden [128=2x64hid, 128cells]
    (halves PE columns and relu-evac volume vs unpaired)
  - b2 via one K=1 ones-row matmul per 8-pair group (fills PSUM [128,512]),
    0.3*A via PE: quad-transposes of a16 + accumulate-matmul with 0.3*I
  - output stage is then a plain PSUM->SBUF copy, splittable DVE/ACT
  - fp32 out DMA on sync queue
"""

import sys

sys.path.insert(0, "/opt/trn_rl_repo")

from contextlib import ExitStack

import numpy as np

import concourse.bass as bass
import concourse.tile as tile
from concourse import bacc, mybir
from concourse._compat import with_exitstack
from concourse.bass_utils import run_bass_kernel_spmd

F32 = mybir.dt.float32
F16 = mybir.dt.float16
BF16 = mybir.dt.bfloat16
AF = mybir.ActivationFunctionType
OP = mybir.AluOpType

H, W, D, HID = 1024, 1024, 32, 64
NCORES = 8
P = 128
JPC = (H // NCORES) * W // P  # 1024 j-positions per partition per core
JPB = 64                      # j's per block (8192 cells)
NBLK = JPC // JPB             # 16 blocks
NPAIR = JPB // 2              # 32 cell-chunk pairs per block
EPS = 1e-8
ALPHA = 0.3
PATTERN_W = 0.1

# engine split for PSUM-evacuation work (tunable)
EVAC_ENG = ["act", "dve", "act", "act", "act", "dve", "act", "act"]  # 8x [128,512] relu
PX_ENG = ["dve", "act", "dve", "act"]    # 4x [68,1024] copy
OUT_ENG = ["act", "dve", "act", "dve"]   # 4x [128,512] f32 copy
ATQ_ENG = ["dve", "act", "dve", "act"]   # 4x [128,512] f16 copy
USE_PE_ALPHA = True  # True: 0.3*A via PE accumulate; False: DVE stt combine


@with_exitstack
def _body(ctx: ExitStack, tc: tile.TileContext, gv, ppv, w1v, b1v, w2v, b2v, ov,
          identv, nblk=NBLK, reps=1):
    nc = tc.nc

    const = ctx.enter_context(tc.tile_pool(name="const", bufs=1))
    apool = ctx.enter_context(tc.tile_pool(name="a", bufs=2))
    epool = ctx.enter_context(tc.tile_pool(name="elw", bufs=2))
    tpool = ctx.enter_context(tc.tile_pool(name="tree", bufs=2))
    spool = ctx.enter_context(tc.tile_pool(name="small", bufs=2))
    xpool = ctx.enter_context(tc.tile_pool(name="x68", bufs=2))
    rhsp = ctx.enter_context(tc.tile_pool(name="rhs", bufs=2))
    hp = ctx.enter_context(tc.tile_pool(name="hsb", bufs=4))
    atp = ctx.enter_context(tc.tile_pool(name="atsb", bufs=2))
    outp = ctx.enter_context(tc.tile_pool(name="osb", bufs=2))
    ps_x = ctx.enter_context(tc.tile_pool(name="psx", bufs=2, space="PSUM"))
    ps_m = ctx.enter_context(tc.tile_pool(name="psm", bufs=2, space="PSUM"))
    ps_n = ctx.enter_context(tc.tile_pool(name="psn", bufs=2, space="PSUM"))
    ps_t = ctx.enter_context(tc.tile_pool(name="pst", bufs=2, space="PSUM"))

    # ---------------- constants ----------------
    idt = const.tile([P, 128], F16, name="idt")
    nc.sync.dma_start(idt[:, :], identv[:, :])

    # block-diagonal [W1;b1] pairs: [68, 128]
    wdiag = const.tile([68, 128], F16, name="wdiag")
    nc.vector.memset(wdiag[:, :], 0.0)
    nc.gpsimd.dma_start(wdiag[0:33, 0:64], w1v[:, :])
    nc.gpsimd.dma_start(wdiag[34:67, 64:128], w1v[:, :])
    b1r = b1v.rearrange("(one h) -> one h", one=1)
    nc.gpsimd.dma_start(wdiag[33:34, 0:64], b1r)
    nc.gpsimd.dma_start(wdiag[67:68, 64:128], b1r)

    # block-diagonal W2 pairs: [128, 64]
    w2diag = const.tile([128, 64], F16, name="w2diag")
    nc.vector.memset(w2diag[:, :], 0.0)
    nc.gpsimd.dma_start(w2diag[0:64, 0:32], w2v[:, :])
    nc.gpsimd.dma_start(w2diag[64:128, 32:64], w2v[:, :])

    # 0.3*I for the a-accumulate matmul
    alphaI = const.tile([P, 128], F16, name="alphaI")
    nc.vector.tensor_scalar(alphaI[:, :], idt[:, :], ALPHA, None, op0=OP.mult)

    # b2 tiled x16 [1, 512] + ones row [1, 128] for the K=1 bias matmul
    b2row = const.tile([1, 512], F16, name="b2row")
    b2r = b2v.rearrange("(one d) -> one d", one=1)
    nc.gpsimd.dma_start(b2row[0:1, 0:32], b2r)
    nc.vector.tensor_copy(b2row[0:1, 32:64], b2row[0:1, 0:32])
    nc.vector.tensor_copy(b2row[0:1, 64:128], b2row[0:1, 0:64])
    nc.vector.tensor_copy(b2row[0:1, 128:256], b2row[0:1, 0:128])
    nc.vector.tensor_copy(b2row[0:1, 256:512], b2row[0:1, 0:256])
    ones1 = const.tile([1, 128], F16, name="ones1")
    nc.vector.memset(ones1[:, :], 1.0)

    # whole-core potentials [128, 1024] fp32 (one DMA)
    ppt = const.tile([P, JPC], F32, name="ppt")
    nc.sync.dma_start(ppt[:, :], ppv[:, :])

    for rep in range(reps):
      for b in range(nblk):
        # ---- load (single HBM read, cast to fp16) ----
        a16 = apool.tile([P, JPB * D], F16, tag="a16")
        a3 = a16[:].rearrange("p (j d) -> p j d", d=D)
        nc.gpsimd.dma_start(a3, gv[:, JPB * b : JPB * (b + 1), :])

        # ---- elementwise (DVE packed) ----
        sq = epool.tile([P, JPB * D], F16, tag="sq")
        nc.vector.tensor_mul(sq[:], a16[:], a16[:])
        cube = epool.tile([P, JPB * D], F16, tag="cube")
        nc.vector.tensor_mul(cube[:], sq[:], a16[:])
        six = epool.tile([P, JPB * D], BF16, tag="six")
        nc.vector.tensor_mul(six[:], cube[:], cube[:])

        # ---- d-reduction add-trees (s2 from sq, s6 from six) ----
        def tree(src, dt16, tag):
            u1 = tpool.tile([P, JPB * 16], dt16, tag=f"{tag}u1")
            u2 = tpool.tile([P, JPB * 8], dt16, tag=f"{tag}u2")
            u3 = tpool.tile([P, JPB * 4], dt16, tag=f"{tag}u3")
            u4 = tpool.tile([P, JPB * 2], dt16, tag=f"{tag}u4")
            s = spool.tile([P, JPB], F32, tag=f"{tag}s")
            s3 = src[:].rearrange("p (j d) -> p j d", d=D)
            v1 = u1[:].rearrange("p (j d) -> p j d", d=16)
            nc.vector.tensor_add(v1, s3[:, :, 0:16], s3[:, :, 16:32])
            v2 = u2[:].rearrange("p (j d) -> p j d", d=8)
            nc.vector.tensor_add(v2, v1[:, :, 0:8], v1[:, :, 8:16])
            v3 = u3[:].rearrange("p (j d) -> p j d", d=4)
            nc.vector.tensor_add(v3, v2[:, :, 0:4], v2[:, :, 4:8])
            v4 = u4[:].rearrange("p (j d) -> p j d", d=2)
            nc.vector.tensor_add(v4, v3[:, :, 0:2], v3[:, :, 2:4])
            nc.vector.tensor_add(
                s[:].rearrange("p (j one) -> p j one", one=1),
                v4[:, :, 0:1], v4[:, :, 1:2])
            return s

        s2 = tree(sq, F16, "s2")
        s6 = tree(six, BF16, "s6")

        # ---- rb = 0.1*sqrt(s2)/(sqrt(s6)+eps), fp32 [p, 64] ----
        cn = spool.tile([P, JPB], F32, tag="cn")
        nc.scalar.activation(cn[:], s2[:], AF.Sqrt)
        c6 = spool.tile([P, JPB], F32, tag="c6")
        nc.scalar.activation(c6[:], s6[:], AF.Sqrt)
        nc.vector.tensor_scalar(c6[:], c6[:], EPS, None, op0=OP.add)
        inv = spool.tile([P, JPB], F32, tag="inv")
        nc.vector.reciprocal(inv[:], c6[:])
        rb = spool.tile([P, JPB], F32, tag="rb")
        nc.vector.scalar_tensor_tensor(
            rb[:], cn[:], PATTERN_W, inv[:], op0=OP.mult, op1=OP.mult)

        # ---- tmp = cube * rb (GPSIMD broadcast; fast path on Q7) ----
        tmp = epool.tile([P, JPB * D], F16, tag="tmp")
        rbb = rb[:].rearrange("p (j one) -> p j one", one=1).broadcast_to(
            (P, JPB, D))
        nc.gpsimd.tensor_mul(
            tmp[:].rearrange("p (j d) -> p j d", d=D),
            cube[:].rearrange("p (j d) -> p j d", d=D), rbb)

        # ---- x-assembly: pairs [128, 68] = [cwp|pot|1 , cwp|pot|1] ----
        x68 = xpool.tile([P, NPAIR * 68], F16, tag="x68")
        x4 = x68[:].rearrange("p (q s) -> p q s", s=68)
        x4s = x4.rearrange("p q (t s) -> p q t s", s=34)
        a4 = a16[:].rearrange("p (q t d) -> p q t d", t=2, d=D)
        t4 = tmp[:].rearrange("p (q t d) -> p q t d", t=2, d=D)
        nc.vector.tensor_add(x4s[:, :, :, 0:D], a4, t4)
        pp4 = ppt[:, JPB * b : JPB * (b + 1)].rearrange(
            "p (q t one) -> p q t one", t=2, one=1)
        nc.vector.tensor_copy(x4s[:, :, :, D : D + 1], pp4)
        nc.vector.tensor_scalar(
            x4s[:, :, :, D + 1 : D + 2], x4s[:, :, :, D : D + 1],
            0.0, 1.0, op0=OP.mult, op1=OP.add)

        osb = outp.tile([P, JPB * D], F32, tag="osb")

        # ---- per 8-pair group ----
        for g in range(4):
            # transposed x-pairs -> SBUF rhs
            px = ps_x.tile([68, 1024], F16, tag="px")
            for i in range(8):
                pair = 8 * g + i
                nc.tensor.transpose(
                    px[0:68, 128 * i : 128 * (i + 1)],
                    x4[:, pair, :], idt[:, :], tile_position=(0, 0))
            rhs = rhsp.tile([68, 1024], F16, tag="rhs")
            if PX_ENG[g] == "dve":
                nc.vector.tensor_copy(rhs[0:68, :], px[0:68, :])
            else:
                nc.scalar.copy(rhs[0:68, :], px[0:68, :])

            # transposed a-quads -> SBUF (for the 0.3*A accumulate)
            if USE_PE_ALPHA:
                pt = ps_t.tile([P, 512], F16, tag="pt")
                for k in range(4):
                    nc.tensor.transpose(
                        pt[:, 128 * k : 128 * (k + 1)],
                        a16[:, 512 * g + 128 * k : 512 * g + 128 * (k + 1)],
                        idt[:, :], tile_position=(0, 0))
                atq = atp.tile([P, 512], F16, tag="atq")
                if ATQ_ENG[g] == "dve":
                    nc.vector.tensor_copy(atq[:, :], pt[:, :])
                else:
                    nc.scalar.copy(atq[:, :], pt[:, :])

            # mm1 + relu evac + mm2 + bias + 0.3*A accumulate
            pn = ps_n.tile([P, 512], F32, tag="pn")
            nc.tensor.matmul(pn[:, :], lhsT=ones1[0:1, :], rhs=b2row[0:1, :],
                             start=True, stop=False)
            for q in range(2):
                pm = ps_m.tile([P, 512], F32, tag="pm")
                for c in range(4):
                    nc.tensor.matmul(
                        pm[:, 128 * c : 128 * (c + 1)],
                        lhsT=wdiag[0:68, :],
                        rhs=rhs[0:68, 512 * q + 128 * c : 512 * q + 128 * (c + 1)],
                        start=True, stop=True)
                h2 = hp.tile([P, 512], F16, tag="h2")
                if EVAC_ENG[2 * g + q] == "act":
                    nc.scalar.activation(h2[:, :], pm[:, :], AF.Relu)
                else:
                    nc.vector.tensor_scalar(h2[:, :], pm[:, :], 0.0, None,
                                            op0=OP.max)
                for c in range(4):
                    nc.tensor.matmul(
                        pn[:, 256 * q + 64 * c : 256 * q + 64 * (c + 1)],
                        lhsT=h2[:, 128 * c : 128 * (c + 1)],
                        rhs=w2diag[:, :], start=False,
                        stop=(not USE_PE_ALPHA and q == 1 and c == 3),
                        skip_group_check=True)

            dst = osb[:, 512 * g : 512 * (g + 1)]
            if USE_PE_ALPHA:
                for k in range(4):
                    nc.tensor.matmul(
                        pn[:, 128 * k : 128 * (k + 1)],
                        lhsT=atq[:, 128 * k : 128 * (k + 1)],
                        rhs=alphaI[:, :], start=False, stop=(k == 3),
                        skip_group_check=True)
                # output evacuation (plain copy)
                if OUT_ENG[g] == "dve":
                    nc.vector.tensor_copy(dst, pn[:, :])
                else:
                    nc.scalar.copy(dst, pn[:, :])
            else:
                nc.vector.scalar_tensor_tensor(
                    dst, a16[:, 512 * g : 512 * (g + 1)], ALPHA, pn[:, :],
                    op0=OP.mult, op1=OP.add)

        nc.sync.dma_start(
            ov[:, JPB * b : JPB * (b + 1), :],
            osb[:].rearrange("p (j d) -> p j d", d=D))


_CACHE = {}


def _build(nblk=NBLK, reps=1):
    if (nblk, reps) in _CACHE:
        return _CACHE[(nblk, reps)]
    nc = bacc.Bacc("TRN2", target_bir_lowering=False, debug=False,
                   enable_asserts=False, num_devices=NCORES)
    jpc = max(nblk * JPB, JPC)
    g = nc.dram_tensor("g", [P, jpc, D], F32, kind="ExternalInput")
    pp = nc.dram_tensor("pp", [P, jpc], F32, kind="ExternalInput")
    w1 = nc.dram_tensor("w1", [D + 1, HID], F32, kind="ExternalInput")
    b1 = nc.dram_tensor("b1", [HID], F32, kind="ExternalInput")
    w2 = nc.dram_tensor("w2", [HID, D], F32, kind="ExternalInput")
    b2 = nc.dram_tensor("b2", [D], F32, kind="ExternalInput")
    out = nc.dram_tensor("out", [P, jpc, D], F32, kind="ExternalOutput")
    ident = nc.inline_tensor(np.eye(P, dtype=np.float16), name="ident")
    with tile.TileContext(nc) as tc:
        _body(tc, g.ap(), pp.ap(), w1.ap(), b1.ap(), w2.ap(), b2.ap(),
              out.ap(), ident.ap(), nblk=nblk, reps=reps)
    nc.compile()
    _CACHE[(nblk, reps)] = nc
    return nc


def kernel(grid_states, potentials, W1, b1, W2, b2):
    nc = _build()
    g = np.asarray(grid_states, dtype=np.float32)
    p = np.asarray(potentials, dtype=np.float32)
    rows = H // NCORES
    in_maps = []
    for c in range(NCORES):
        in_maps.append({
            "g": np.ascontiguousarray(
                g[c * rows : (c + 1) * rows].reshape(P, JPC, D)),
            "pp": np.ascontiguousarray(
                p[c * rows : (c + 1) * rows].reshape(P, JPC)),
            "w1": np.asarray(W1, dtype=np.float32),
            "b1": np.asarray(b1, dtype=np.float32),
            "w2": np.asarray(W2, dtype=np.float32),
            "b2": np.asarray(b2, dtype=np.float32),
        })
    import os
    trace = bool(int(os.environ.get("BENG_TRACE", "0")))
    res = run_bass_kernel_spmd(nc, in_maps, core_ids=list(range(NCORES)),
                               trace=trace)
    _CACHE["last_res"] = res
    outs = [res.results[c]["out"].reshape(rows, W, D) for c in range(NCORES)]
    return np.concatenate(outs, axis=0)


if __name__ == "__main__":
    rng = np.random.default_rng(0)
    gs = rng.standard_normal((H, W, D), dtype=np.float32)
    po = rng.random((H, W), dtype=np.float32)
    W1a = rng.standard_normal((D + 1, HID), dtype=np.float32) * 0.1
    b1a = rng.standard_normal((HID,), dtype=np.float32) * 0.1
    W2a = rng.standard_normal((HID, D), dtype=np.float32) * 0.1
    b2a = rng.standard_normal((D,), dtype=np.float32) * 0.1
    o = kernel(gs, po, W1a, b1a, W2a, b2a)
    print(o.shape, o.dtype)
